# revision 8
# baseline (speedup 1.0000x reference)
"""Trainium2 Bass kernel for nn_CrossAttnMem (channel self-attention + batch-flattened
cross attention) — N-sharded, transfer-minimal version.

Both attention paths factor through rank-64 Gram matrices, so the only O(N)
work is (a) the Gram contraction E^T E over tokens and (b) the final
out = Eu @ M output matmuls.  Everything between is tiny [64,*] algebra.

Sharding: each of the 8 cores owns a 512-token slice of all 8 batches.
  1. Each core computes partial Grams (Eu_bu^T El_b, Eu_b^T Eu_b) over its
     slice -> one [64,1280] f32 AllReduce gives every core the full Grams.
  2. Weights arrive as a per-core [64,1024] fp16 slab -> AllGather.
  3. The attention algebra (InstanceNorm stats computed algebraically from
     the Grams, softmax denominator folded into the output projections) is
     replicated on every core, producing per-batch [64,64] matrices
     M_{b,bu} (cross) and Weff_b (self).
  4. Each core emits out[g, its-slice] = Eu @ M / Eu @ Weff exactly — no
     partial sums on the host.

Wall-clock here is dominated by the axon tunnel, so all I/O is fp16 and the
per-call total is ~9 MB instead of the ~104 MB of the batch-replicated
baseline.  The PJRT executable is compiled once and cached; output donation
buffers are created on-device (never shipped).
"""

import numpy as np

H = 8
C = 64
HC = 512
N = 4096
B = 4
EPS = 1e-5
NC = 8           # cores
NT = 4           # n-tiles of 128 per core
CNT_CROSS = float(HC * B * HC)
CNT_SELF = float(C * C)

_CACHE = {}


# --------------------------------------------------------------------------
# Bass kernel
# --------------------------------------------------------------------------
def _build():
    import concourse.bass as bass  # noqa: F401
    import concourse.mybir as mybir
    import concourse.tile as tile
    from concourse import bacc

    dt = mybir.dt
    f32 = dt.float32
    f16 = dt.float16
    AF = mybir.ActivationFunctionType

    nc = bacc.Bacc("TRN2", target_bir_lowering=False, debug=False,
                   num_devices=NC)

    es_d = nc.dram_tensor("es", [128, NT * 8 * 64], f16,
                          kind="ExternalInput").ap()
    wsl_d = nc.dram_tensor("wsl", [64, 1024], f16, kind="ExternalInput").ap()
    out_d = nc.dram_tensor("out", [NT, 8, 128, 64], f16,
                           kind="ExternalOutput").ap()

    RG = [list(range(NC))]

    with tile.TileContext(nc) as tc:
        with (
            tc.tile_pool(name="dram", bufs=1, space="DRAM") as dram,
            tc.tile_pool(name="wt", bufs=1) as wt,
            tc.tile_pool(name="emb", bufs=1) as embp,
            tc.tile_pool(name="wrk", bufs=1) as wrk,
        ):
            # ---------- Phase A: load + AllGather weights ----------
            es = embp.tile([128, NT * 8 * 64], f16)
            nc.sync.dma_start(es[:], es_d)

            ag_in = dram.tile([64, 1024], f16)
            ag_out = dram.tile([512, 1024], f16, addr_space="Shared")
            nc.gpsimd.dma_start(ag_in[:], wsl_d)
            nc.gpsimd.collective_compute(
                "AllGather", mybir.AluOpType.bypass, replica_groups=RG,
                ins=[ag_in.opt()], outs=[ag_out.opt()])

            wqu = wt.tile([64, 512], f16)
            wku = wt.tile([64, 512], f16)
            wvut = wt.tile([64, 512], f16)
            woup = wt.tile([64, 512], f16)
            wq = wt.tile([64, 512], f16)
            wk = wt.tile([64, 512], f16)
            wvt = wt.tile([128, 256], f16)
            wo = wt.tile([128, 256], f16)
            pq16 = wt.tile([64, 64], f16)
            pk16 = wt.tile([64, 64], f16)
            uqk16 = wt.tile([64, 2], f16)
            id128 = wt.tile([128, 128], f16)
            nc.sync.dma_start(wqu[:], ag_out[0:64, 0:512])
            nc.sync.dma_start(pq16[:], ag_out[0:64, 512:576])
            nc.sync.dma_start(pk16[:], ag_out[0:64, 576:640])
            nc.sync.dma_start(uqk16[:], ag_out[0:64, 640:642])
            nc.sync.dma_start(id128[0:64, :], ag_out[0:64, 704:832])
            nc.sync.dma_start(id128[64:128, :], ag_out[0:64, 832:960])
            nc.sync.dma_start(wku[:], ag_out[64:128, 0:512])
            nc.sync.dma_start(wvut[:], ag_out[128:192, 0:512])
            nc.sync.dma_start(woup[:], ag_out[192:256, 0:512])
            nc.sync.dma_start(wq[:], ag_out[256:320, 0:512])
            nc.sync.dma_start(wk[:], ag_out[320:384, 0:512])
            nc.sync.dma_start(wvt[0:64, :], ag_out[384:448, 0:256])
            nc.sync.dma_start(wvt[64:128, :], ag_out[384:448, 256:512])
            nc.sync.dma_start(wo[0:64, :], ag_out[448:512, 0:256])
            nc.sync.dma_start(wo[64:128, :], ag_out[448:512, 256:512])

            # f32 casts for the stats algebra
            pq32 = wrk.tile([64, 64], f32)
            pk32 = wrk.tile([64, 64], f32)
            uqk32 = wrk.tile([64, 2], f32)
            id8 = wrk.tile([8, 8], f32)
            nc.scalar.copy(pq32[:], pq16[:])
            nc.scalar.copy(pk32[:], pk16[:])
            nc.scalar.copy(uqk32[:], uqk16[:])
            nc.scalar.copy(id8[:], id128[0:8, 0:8])
            onesr = wrk.tile([1, 128], f32)
            onesc = wrk.tile([64, 1], f32)
            nc.vector.memset(onesr[:], 1.0)
            nc.vector.memset(onesc[:], 1.0)

            def eslice(t, g):
                return es[:, (t * 8 + g) * 64:(t * 8 + g + 1) * 64]

            # ---------- Phase B: local Grams -> AllReduce ----------
            G = wrk.tile([64, 1280], f32)
            G16 = wrk.tile([64, 1280], f16)
            esuT = embp.tile([64, NT * 4 * 128], f16)
            with (
                tc.tile_pool(name="gps", bufs=1, space="PSUM") as gps,
                tc.tile_pool(name="tps", bufs=2, space="PSUM") as tps,
            ):
                g_ps = gps.tile([64, 1280], f32)
                for b in range(4):
                    for bu in range(4):
                        col = (b * 4 + bu) * 64
                        for t in range(NT):
                            nc.tensor.matmul(
                                g_ps[:, col:col + 64], eslice(t, 4 + bu),
                                eslice(t, b), start=(t == 0), stop=(t == 3))
                for b in range(4):
                    col = 1024 + b * 64
                    for t in range(NT):
                        nc.tensor.matmul(
                            g_ps[:, col:col + 64], eslice(t, 4 + b),
                            eslice(t, 4 + b), start=(t == 0), stop=(t == 3))
                gl = wrk.tile([64, 1280], f32, tag="gl")
                nc.scalar.copy(gl[:], g_ps[:])
                ar_in = dram.tile([64, 1280], f32)
                ar_out = dram.tile([64, 1280], f32, addr_space="Shared")
                nc.gpsimd.dma_start(ar_in[:], gl[:])
                nc.gpsimd.collective_compute(
                    "AllReduce", mybir.AluOpType.add, replica_groups=RG,
                    ins=[ar_in.opt()], outs=[ar_out.opt()])
                nc.sync.dma_start(G[:], ar_out[:])
                nc.scalar.copy(G16[:], G[:])

                # transposes of the Eu slices for the output matmuls
                for t in range(NT):
                    for bu in range(4):
                        tp = tps.tile([64, 128], f16, tag="tp")
                        nc.tensor.transpose(tp[:], eslice(t, 4 + bu), id128[:])
                        col = (t * 4 + bu) * 128
                        nc.scalar.copy(esuT[:, col:col + 128], tp[:])

            def Gt32(b, bu):
                col = (b * 4 + bu) * 64
                return G[:, col:col + 64]

            def Gt16(b, bu):
                col = (b * 4 + bu) * 64
                return G16[:, col:col + 64]

            def Guu16(b):
                col = 1024 + b * 64
                return G16[:, col:col + 64]

            # ---------- Phase D: cross-path inorm stats (per b) ----------
            bcv = wrk.tile([128, 8], f32)        # per b: (rstd, -mean*rstd)
            with tc.tile_pool(name="stp", bufs=1, space="PSUM") as stp:
                st_ps = stp.tile([1, 8], f32)
                pairs8 = wrk.tile([1, 8], f32, tag="pairs8")
                for b in range(4):
                    g01 = wrk.tile([64, 64], f32, tag="g01")
                    g23 = wrk.tile([64, 64], f32, tag="g23")
                    gsum = wrk.tile([64, 64], f32, tag="gsum")
                    nc.vector.tensor_add(g01[:], Gt32(b, 0), Gt32(b, 1))
                    nc.vector.tensor_add(g23[:], Gt32(b, 2), Gt32(b, 3))
                    nc.vector.tensor_add(gsum[:], g01[:], g23[:])
                    v1_ps = stp.tile([64, 1], f32, tag="v1")
                    nc.tensor.matmul(v1_ps[:], gsum[:], uqk32[:, 1:2])
                    v1 = wrk.tile([64, 1], f32, tag="v1sb")
                    nc.scalar.copy(v1[:], v1_ps[:])
                    nc.tensor.matmul(st_ps[:, 2 * b:2 * b + 1], v1[:],
                                     uqk32[:, 0:1])

                    z_ps = stp.tile([64, 256], f32, tag="z")
                    for bu in range(4):
                        nc.tensor.matmul(z_ps[:, bu * 64:(bu + 1) * 64],
                                         pk32[:], Gt32(b, bu))
                    z_sb = wrk.tile([64, 256], f32, tag="zsb")
                    nc.scalar.copy(z_sb[:], z_ps[:])
                    y_ps = stp.tile([64, 64], f32, tag="y")
                    for bu in range(4):
                        nc.tensor.matmul(y_ps[:], Gt32(b, bu),
                                         z_sb[:, bu * 64:(bu + 1) * 64],
                                         start=(bu == 0), stop=(bu == 3))
                    mq = wrk.tile([64, 64], f32, tag="mq")
                    nc.vector.tensor_mul(mq[:], pq32[:], y_ps[:])
                    mv = wrk.tile([64, 1], f32, tag="mv")
                    nc.vector.reduce_sum(mv[:], mq[:],
                                         axis=mybir.AxisListType.X)
                    nc.tensor.matmul(st_ps[:, 2 * b + 1:2 * b + 2], mv[:],
                                     onesc[:])

                for b in range(4):
                    mean = wrk.tile([1, 1], f32, tag="c0")
                    ex2 = wrk.tile([1, 1], f32, tag="c1")
                    m2 = wrk.tile([1, 1], f32, tag="c2")
                    var = wrk.tile([1, 1], f32, tag="c3")
                    std = wrk.tile([1, 1], f32, tag="c4")
                    rstd = wrk.tile([1, 1], f32, tag="c5")
                    nb = wrk.tile([1, 1], f32, tag="c6")
                    nc.scalar.mul(mean[:], st_ps[:, 2 * b:2 * b + 1],
                                  1.0 / CNT_CROSS)
                    nc.scalar.mul(ex2[:], st_ps[:, 2 * b + 1:2 * b + 2],
                                  1.0 / CNT_CROSS)
                    nc.scalar.square(m2[:], mean[:])
                    nc.vector.tensor_sub(var[:], ex2[:], m2[:])
                    nc.vector.tensor_scalar_add(var[:], var[:], EPS)
                    nc.scalar.activation(std[:], var[:], AF.Sqrt)
                    nc.vector.reciprocal(rstd[:], std[:])
                    nc.vector.tensor_mul(nb[:], mean[:], rstd[:])
                    nc.scalar.copy(pairs8[:, 2 * b:2 * b + 1], rstd[:])
                    nc.scalar.mul(pairs8[:, 2 * b + 1:2 * b + 2], nb[:], -1.0)
                bc_ps = stp.tile([128, 8], f32, tag="bc")
                nc.tensor.matmul(bc_ps[:], onesr[:], pairs8[:])
                nc.scalar.copy(bcv[:], bc_ps[:])

            # ---------- Phase E: self path (per b) ----------
            Weff = wrk.tile([64, 256], f16)
            for b in range(4):
                with tc.tile_pool(name=f"sf{b}", bufs=1, space="PSUM") as sfp:
                    t1_ps = sfp.tile([64, 512], f32, tag="t1")
                    nc.tensor.matmul(t1_ps[:], Guu16(b), wku[:])
                    t1 = wrk.tile([64, 512], f16, tag="t1sb")
                    nc.scalar.copy(t1[:], t1_ps[:])
                    sc_ps = sfp.tile([64, 512], f32, tag="sc")
                    for h in range(H):
                        hb = slice(h * 64, (h + 1) * 64)
                        nc.tensor.matmul(sc_ps[:, hb], wqu[:, hb], t1[:, hb])
                    ss = wrk.tile([64, 16], f32, tag="ss")
                    dump = wrk.tile([64, 64], f32, tag="dump")
                    for h in range(H):
                        hb = slice(h * 64, (h + 1) * 64)
                        nc.scalar.activation(dump[:], sc_ps[:, hb], AF.Copy,
                                             accum_out=ss[:, h:h + 1])
                        nc.scalar.activation(dump[:], sc_ps[:, hb], AF.Square,
                                             accum_out=ss[:, 8 + h:9 + h])
                    tot_ps = sfp.tile([8, 2], f32, tag="tot")
                    nc.tensor.matmul(tot_ps[:, 0:1], ss[:, 0:8], onesc[:])
                    nc.tensor.matmul(tot_ps[:, 1:2], ss[:, 8:16], onesc[:])
                    mean = wrk.tile([8, 1], f32, tag="s0")
                    ex2 = wrk.tile([8, 1], f32, tag="s1")
                    m2 = wrk.tile([8, 1], f32, tag="s2")
                    var = wrk.tile([8, 1], f32, tag="s3")
                    std = wrk.tile([8, 1], f32, tag="s4")
                    pairs = wrk.tile([8, 2], f32, tag="s5")
                    rstd = wrk.tile([8, 1], f32, tag="s6")
                    nb = wrk.tile([8, 1], f32, tag="s7")
                    nc.scalar.mul(mean[:], tot_ps[:, 0:1], 1.0 / CNT_SELF)
                    nc.scalar.mul(ex2[:], tot_ps[:, 1:2], 1.0 / CNT_SELF)
                    nc.scalar.square(m2[:], mean[:])
                    nc.vector.tensor_sub(var[:], ex2[:], m2[:])
                    nc.vector.tensor_scalar_add(var[:], var[:], EPS)
                    nc.scalar.activation(std[:], var[:], AF.Sqrt)
                    nc.vector.reciprocal(rstd[:], std[:])
                    nc.vector.tensor_mul(nb[:], mean[:], rstd[:])
                    nc.scalar.copy(pairs[:, 0:1], rstd[:])
                    nc.scalar.mul(pairs[:, 1:2], nb[:], -1.0)
                    rT_ps = sfp.tile([1, 8], f32, tag="rT")
                    nT_ps = sfp.tile([1, 8], f32, tag="nT")
                    nc.tensor.transpose(rT_ps[:], pairs[:, 0:1], id8[:])
                    nc.tensor.transpose(nT_ps[:], pairs[:, 1:2], id8[:])
                    rn = wrk.tile([1, 16], f32, tag="rn")
                    nc.scalar.copy(rn[:, 0:8], rT_ps[:])
                    nc.scalar.copy(rn[:, 8:16], nT_ps[:])
                    sb_ps = sfp.tile([64, 16], f32, tag="sb")
                    nc.tensor.matmul(sb_ps[:], onesr[0:1, 0:64], rn[:])
                    sbm = wrk.tile([64, 16], f32, tag="sbm")
                    nc.scalar.copy(sbm[:], sb_ps[:])
                    Es = wrk.tile([64, 512], f16, tag="es16")
                    er = wrk.tile([64, 8], f32, tag="er")
                    for h in range(H):
                        hb = slice(h * 64, (h + 1) * 64)
                        nc.scalar.activation(Es[:, hb], sc_ps[:, hb], AF.Exp,
                                             scale=sbm[:, h:h + 1],
                                             bias=sbm[:, 8 + h:9 + h],
                                             accum_out=er[:, h:h + 1])
                    rec = wrk.tile([64, 8], f32, tag="rec")
                    nc.vector.reciprocal(rec[:], er[:])
                    wosc = wrk.tile([64, 512], f16, tag="wosc")
                    for h in range(H):
                        hb = slice(h * 64, (h + 1) * 64)
                        nc.vector.tensor_scalar_mul(wosc[:, hb], woup[:, hb],
                                                    rec[:, h:h + 1])
                    ys_ps = sfp.tile([64, 512], f32, tag="ys")
                    for h in range(H):
                        hb = slice(h * 64, (h + 1) * 64)
                        nc.tensor.matmul(ys_ps[:, hb], Es[:, hb], wosc[:, hb])
                    ys = wrk.tile([64, 512], f16, tag="yssb")
                    nc.scalar.copy(ys[:], ys_ps[:])
                    we_ps = sfp.tile([64, 64], f32, tag="we")
                    for h in range(H):
                        hb = slice(h * 64, (h + 1) * 64)
                        nc.tensor.matmul(we_ps[:], wvut[:, hb], ys[:, hb],
                                         start=(h == 0), stop=(h == 7))
                    nc.scalar.copy(Weff[:, b * 64:(b + 1) * 64], we_ps[:])

            # ---------- Phase F/G: cross path (per b) ----------
            M = wrk.tile([64, 1024], f16)
            for b in range(4):
                with (
                    tc.tile_pool(name=f"cu{b}", bufs=1, space="PSUM") as cup,
                    tc.tile_pool(name=f"cs{b}", bufs=1, space="PSUM") as csp,
                    tc.tile_pool(name=f"cp{b}", bufs=2, space="PSUM") as cpp,
                    tc.tile_pool(name=f"ce{b}", bufs=1) as cep,
                ):
                    U16 = cep.tile([64, 2048], f16, tag="u16")
                    for bu in range(4):
                        u_ps = cup.tile([64, 512], f32, tag="u")
                        nc.tensor.matmul(u_ps[:], Gt16(b, bu), wk[:])
                        nc.scalar.copy(U16[:, bu * 512:(bu + 1) * 512],
                                       u_ps[:])
                    E = cep.tile([128, 4 * 2048], f16, tag="E")
                    r = wrk.tile([128, 4], f32, tag="r")
                    for dc in range(4):
                        s_ps = csp.tile([128, 2048], f32, tag="s")
                        for bu in range(4):
                            nc.tensor.matmul(
                                s_ps[:, bu * 512:(bu + 1) * 512],
                                wq[:, dc * 128:(dc + 1) * 128],
                                U16[:, bu * 512:(bu + 1) * 512])
                        nc.scalar.activation(
                            E[:, dc * 2048:(dc + 1) * 2048], s_ps[:], AF.Exp,
                            scale=bcv[:, 2 * b:2 * b + 1],
                            bias=bcv[:, 2 * b + 1:2 * b + 2],
                            accum_out=r[:, dc:dc + 1])
                    rec = wrk.tile([128, 4], f32, tag="rr")
                    nc.vector.reciprocal(rec[:], r[:])
                    wosc2 = wrk.tile([128, 256], f16, tag="wosc2")
                    for dc in range(4):
                        nc.vector.tensor_scalar_mul(
                            wosc2[:, dc * 64:(dc + 1) * 64],
                            wo[:, dc * 64:(dc + 1) * 64], rec[:, dc:dc + 1])
                    P1 = cep.tile([128, 1024], f16, tag="p1")
                    for bu in range(4):
                        for kvc in range(4):
                            p_ps = cpp.tile([128, 64], f32, tag="p")
                            for dc in range(4):
                                off = dc * 2048 + bu * 512 + kvc * 128
                                nc.tensor.matmul(
                                    p_ps[:], E[:, off:off + 128],
                                    wosc2[:, dc * 64:(dc + 1) * 64],
                                    start=(dc == 0), stop=(dc == 3))
                            col = (bu * 4 + kvc) * 64
                            nc.scalar.copy(P1[:, col:col + 64], p_ps[:])
                    for bu in range(4):
                        m_ps = cup.tile([64, 64], f32, tag="m")
                        for kvc in range(4):
                            col = (bu * 4 + kvc) * 64
                            nc.tensor.matmul(m_ps[:],
                                             wvt[:, kvc * 64:(kvc + 1) * 64],
                                             P1[:, col:col + 64],
                                             start=(kvc == 0), stop=(kvc == 3))
                        nc.scalar.copy(M[:, (b * 4 + bu) * 64:
                                          (b * 4 + bu + 1) * 64], m_ps[:])

            # ---------- Phase H: outputs ----------
            with (
                tc.tile_pool(name="op", bufs=2, space="PSUM") as op,
                tc.tile_pool(name="ob", bufs=4) as ob,
            ):
                for t in range(NT):
                    for b in range(4):
                        o_ps = op.tile([128, 64], f32, tag="o")
                        for bu in range(4):
                            ecol = (t * 4 + bu) * 128
                            nc.tensor.matmul(
                                o_ps[:], esuT[:, ecol:ecol + 128],
                                M[:, (b * 4 + bu) * 64:(b * 4 + bu + 1) * 64],
                                start=(bu == 0), stop=(bu == 3))
                        o16 = ob.tile([128, 64], f16, tag="o16")
                        nc.scalar.copy(o16[:], o_ps[:])
                        nc.sync.dma_start(out_d[t, b], o16[:])
                        ou_ps = op.tile([128, 64], f32, tag="ou")
                        ecol = (t * 4 + b) * 128
                        nc.tensor.matmul(ou_ps[:], esuT[:, ecol:ecol + 128],
                                         Weff[:, b * 64:(b + 1) * 64])
                        ou16 = ob.tile([128, 64], f16, tag="ou16")
                        nc.vector.tensor_copy(ou16[:], ou_ps[:])
                        nc.sync.dma_start(out_d[t, 4 + b], ou16[:])
    nc.compile()
    return nc


# --------------------------------------------------------------------------
# Cached PJRT runner (replaces run_bass_kernel_spmd's per-call jit re-trace)
# --------------------------------------------------------------------------
class _Results:
    def __init__(self, results):
        self.results = results
        self.exec_time_ns = None
        self.mean_exec_time_ns = None
        self.max_exec_time_core_id = None


def _make_runner(nc):
    import jax
    import jax.numpy as jnp
    from jax.sharding import Mesh, NamedSharding, PartitionSpec
    try:
        from jax.sharding import shard_map
    except ImportError:
        from jax.experimental.shard_map import shard_map
    import concourse.mybir as mybir
    from concourse.bass2jax import (_bass_exec_p, fast_dispatch_compile,
                                    install_neuronx_cc_hook,
                                    partition_id_tensor)

    install_neuronx_cc_hook()

    pname = nc.partition_id_tensor.name if nc.partition_id_tensor else None
    in_names, out_names, out_avals = [], [], []
    in_shapes = []
    for alloc in nc.m.functions[0].allocations:
        if not isinstance(alloc, mybir.MemoryLocationSet):
            continue
        name = alloc.memorylocations[0].name
        shape = tuple(alloc.tensor_shape) if alloc.tensor_shape else None
        dtype = mybir.dt.np(alloc.dtype) if alloc.dtype is not None else None
        if alloc.kind == "ExternalInput" and name != pname:
            in_names.append(name)
            in_shapes.append((shape, dtype))
        elif alloc.kind == "ExternalOutput":
            out_names.append(name)
            out_avals.append(jax.core.ShapedArray(shape, dtype))
    n_params = len(in_names)
    n_outs = len(out_avals)
    all_in_names = in_names + out_names + ([pname] if pname else [])

    def _body(*args):
        operands = list(args)
        if pname:
            operands.append(partition_id_tensor())
        outs = _bass_exec_p.bind(
            *operands, out_avals=tuple(out_avals),
            in_names=tuple(all_in_names), out_names=tuple(out_names),
            lowering_input_output_aliases=(), sim_require_finite=True,
            sim_require_nnan=True, nc=nc)
        return tuple(outs)

    devices = jax.devices()[:NC]
    mesh = Mesh(np.asarray(devices), ("core",))
    in_specs = (PartitionSpec("core"),) * (n_params + n_outs)
    out_specs = (PartitionSpec("core"),) * n_outs
    donate = tuple(range(n_params, n_params + n_outs))
    fn = shard_map(_body, mesh=mesh, in_specs=in_specs, out_specs=out_specs,
                   check_rep=False)

    structs = [jax.ShapeDtypeStruct((NC * s[0], *s[1:]), d)
               for (s, d) in in_shapes]
    structs += [jax.ShapeDtypeStruct((NC * a.shape[0], *a.shape[1:]), a.dtype)
                for a in out_avals]
    compiled = fast_dispatch_compile(
        lambda: jax.jit(fn, donate_argnums=donate, keep_unused=True)
        .lower(*structs).compile())

    zshard = tuple([NamedSharding(mesh, PartitionSpec("core"))] * n_outs)
    zfn = jax.jit(
        lambda: tuple(jnp.zeros((NC * a.shape[0], *a.shape[1:]), a.dtype)
                      for a in out_avals),
        out_shardings=zshard)
    state = {"donate": None}

    def run(in_maps):
        concat_in = [np.concatenate([m[name] for m in in_maps], axis=0)
                     for name in in_names]
        # The kernel writes every output element, so the donated output
        # buffers' contents are irrelevant: recycle the previous call's
        # (already fetched) outputs instead of shipping/creating zeros.
        donate_bufs = state["donate"] if state["donate"] is not None else zfn()
        out_arrs = compiled(*concat_in, *donate_bufs)
        fetched = [np.asarray(a) for a in out_arrs]
        state["donate"] = out_arrs
        results = []
        for c in range(NC):
            results.append(
                {name: fetched[i].reshape(NC, *out_avals[i].shape)[c]
                 for i, name in enumerate(out_names)})
        return _Results(results)

    return run


def run_on_device(in_maps, **kwargs):
    kwargs.pop("trace", None)
    if "run" not in _CACHE:
        _CACHE["run"] = _make_runner(_build())
    return _CACHE["run"](in_maps)


# --------------------------------------------------------------------------
# Host-side prep / gather
# --------------------------------------------------------------------------
def _prep_inputs(emb, W_qu, W_ku, W_vu, W_ql2u, W_kl2u, W_vl2u, W_out_u,
                 W_out_l2u):
    emb = np.asarray(emb, np.float32)
    # es[c]: [128, NT*8*64] fp16, block (t, g) at cols (t*8+g)*64
    es_all = (emb.reshape(8, NC, NT, 128, C).transpose(1, 3, 2, 0, 4)
              .reshape(NC, 128, NT * 8 * C)).astype(np.float16)

    w_ou = W_out_u.reshape(C, H, C)
    wvut = np.concatenate([W_vu[:, h * 64:(h + 1) * 64].T for h in range(H)],
                          axis=1)
    woup = np.concatenate([w_ou[:, h, :] for h in range(H)], axis=1)
    wvt_sb = (W_vl2u.T.reshape(4, 128, 64).transpose(1, 0, 2)
              .reshape(128, 256))
    wo_sb = (W_out_l2u.reshape(4, 128, 64).transpose(1, 0, 2)
             .reshape(128, 256))
    pq = W_ql2u @ W_ql2u.T
    pk = W_kl2u @ W_kl2u.T
    id128 = np.eye(128, dtype=np.float32)

    slabs = np.zeros((NC, 64, 1024), np.float32)
    slabs[0, :, 0:512] = W_qu
    slabs[0, :, 512:576] = pq
    slabs[0, :, 576:640] = pk
    slabs[0, :, 640] = W_ql2u.sum(axis=1)
    slabs[0, :, 641] = W_kl2u.sum(axis=1)
    slabs[0, :, 704:832] = id128[0:64]
    slabs[0, :, 832:960] = id128[64:128]
    slabs[1, :, 0:512] = W_ku
    slabs[2, :, 0:512] = wvut
    slabs[3, :, 0:512] = woup
    slabs[4, :, 0:512] = W_ql2u
    slabs[5, :, 0:512] = W_kl2u
    slabs[6, :, 0:256] = wvt_sb[0:64]
    slabs[6, :, 256:512] = wvt_sb[64:128]
    slabs[7, :, 0:256] = wo_sb[0:64]
    slabs[7, :, 256:512] = wo_sb[64:128]
    slabs16 = slabs.astype(np.float16)

    return [{"es": np.ascontiguousarray(es_all[c]),
             "wsl": np.ascontiguousarray(slabs16[c])} for c in range(NC)]


def kernel(emb, pseudo_label, pseudo_prob_map, W_qu, W_ku, W_vu, W_ql2u,
           W_kl2u, W_vl2u, W_out_u, W_out_l2u, using_SMem, _bass_results=None,
           **_unused):
    del pseudo_label, pseudo_prob_map, using_SMem
    to32 = lambda x: np.asarray(x, np.float32)
    in_maps = _prep_inputs(to32(emb), to32(W_qu), to32(W_ku), to32(W_vu),
                           to32(W_ql2u), to32(W_kl2u), to32(W_vl2u),
                           to32(W_out_u), to32(W_out_l2u))
    if _bass_results is None:
        _bass_results = run_on_device(in_maps).results
    # out[c]: [NT, 8, 128, 64] fp16 -> full[g, c*512 + t*128 + p, k]
    stacked = np.stack([_bass_results[c]["out"] for c in range(NC)])
    out = (stacked.astype(np.float32).transpose(2, 0, 1, 3, 4)
           .reshape(8, N, C))
    return np.ascontiguousarray(out)


# revision 11
# speedup vs baseline: 1.0557x; 1.0557x over previous
"""Trainium2 Bass kernel for nn_CrossAttnMem (channel self-attention + batch-flattened
cross attention) — N-sharded, transfer-minimal version.

Both attention paths factor through rank-64 Gram matrices, so the only O(N)
work is (a) the Gram contraction E^T E over tokens and (b) the final
out = Eu @ M output matmuls.  Everything between is tiny [64,*] algebra.

Sharding: each of the 8 cores owns a 512-token slice of all 8 batches.
  1. Each core computes partial Grams (Eu_bu^T El_b, Eu_b^T Eu_b) over its
     slice -> one [64,1280] f32 AllReduce gives every core the full Grams.
  2. Weights arrive as a per-core [64,1024] fp16 slab -> AllGather.
  3. The attention algebra (InstanceNorm stats computed algebraically from
     the Grams, softmax denominator folded into the output projections) is
     replicated on every core, producing per-batch [64,64] matrices
     M_{b,bu} (cross) and Weff_b (self).
  4. Each core emits out[g, its-slice] = Eu @ M / Eu @ Weff exactly — no
     partial sums on the host.

Wall-clock here is dominated by the axon tunnel, so all I/O is fp16 and the
per-call total is ~9 MB instead of the ~104 MB of the batch-replicated
baseline.  The PJRT executable is compiled once and cached; output donation
buffers are created on-device (never shipped).
"""

import numpy as np

H = 8
C = 64
HC = 512
N = 4096
B = 4
EPS = 1e-5
NC = 8           # cores
NT = 4           # n-tiles of 128 per core
CNT_CROSS = float(HC * B * HC)
CNT_SELF = float(C * C)

_CACHE = {}


# --------------------------------------------------------------------------
# Bass kernel
# --------------------------------------------------------------------------
def _build():
    import concourse.bass as bass  # noqa: F401
    import concourse.mybir as mybir
    import concourse.tile as tile
    from concourse import bacc

    dt = mybir.dt
    f32 = dt.float32
    f16 = dt.float16
    AF = mybir.ActivationFunctionType

    nc = bacc.Bacc("TRN2", target_bir_lowering=False, debug=False,
                   num_devices=NC)

    # single packed input: cols 0:2048 = emb slice, cols 2048:2560 = the
    # [64,1024] weight slab stored as row 2r+j <- wsl[r, 512j:512(j+1)]
    xin_d = nc.dram_tensor("xin", [128, NT * 8 * 64 + 512], f16,
                           kind="ExternalInput").ap()
    out_d = nc.dram_tensor("out", [NT, 8, 128, 64], f16,
                           kind="ExternalOutput").ap()

    RG = [list(range(NC))]

    with tile.TileContext(nc) as tc:
        with (
            tc.tile_pool(name="dram", bufs=1, space="DRAM") as dram,
            tc.tile_pool(name="wt", bufs=1) as wt,
            tc.tile_pool(name="emb", bufs=1) as embp,
            tc.tile_pool(name="wrk", bufs=1) as wrk,
        ):
            # ---------- Phase A: load + AllGather weights ----------
            es = embp.tile([128, NT * 8 * 64], f16)
            nc.sync.dma_start(es[:], xin_d[:, 0:2048])

            ag_in = dram.tile([64, 1024], f16)
            ag_out = dram.tile([512, 1024], f16, addr_space="Shared")
            nc.gpsimd.dma_start(ag_in[0:64, 0:512], xin_d[0:128:2, 2048:2560])
            nc.gpsimd.dma_start(ag_in[0:64, 512:1024],
                                xin_d[1:128:2, 2048:2560])
            nc.gpsimd.collective_compute(
                "AllGather", mybir.AluOpType.bypass, replica_groups=RG,
                ins=[ag_in.opt()], outs=[ag_out.opt()])

            wqu = wt.tile([64, 512], f16)
            wku = wt.tile([64, 512], f16)
            wvut = wt.tile([64, 512], f16)
            woup = wt.tile([64, 512], f16)
            wq = wt.tile([64, 512], f16)
            wk = wt.tile([64, 512], f16)
            wvt = wt.tile([128, 256], f16)
            wo = wt.tile([128, 256], f16)
            pq16 = wt.tile([64, 64], f16)
            pk16 = wt.tile([64, 64], f16)
            uqk16 = wt.tile([64, 2], f16)
            id128 = wt.tile([128, 128], f16)
            nc.sync.dma_start(wqu[:], ag_out[0:64, 0:512])
            nc.sync.dma_start(pq16[:], ag_out[0:64, 512:576])
            nc.sync.dma_start(pk16[:], ag_out[0:64, 576:640])
            nc.sync.dma_start(uqk16[:], ag_out[0:64, 640:642])
            nc.sync.dma_start(id128[0:64, :], ag_out[0:64, 704:832])
            nc.sync.dma_start(id128[64:128, :], ag_out[0:64, 832:960])
            nc.sync.dma_start(wku[:], ag_out[64:128, 0:512])
            nc.sync.dma_start(wvut[:], ag_out[128:192, 0:512])
            nc.sync.dma_start(woup[:], ag_out[192:256, 0:512])
            nc.sync.dma_start(wq[:], ag_out[256:320, 0:512])
            nc.sync.dma_start(wk[:], ag_out[320:384, 0:512])
            nc.sync.dma_start(wvt[0:64, :], ag_out[384:448, 0:256])
            nc.sync.dma_start(wvt[64:128, :], ag_out[384:448, 256:512])
            nc.sync.dma_start(wo[0:64, :], ag_out[448:512, 0:256])
            nc.sync.dma_start(wo[64:128, :], ag_out[448:512, 256:512])

            # f32 casts for the stats algebra
            pq32 = wrk.tile([64, 64], f32)
            pk32 = wrk.tile([64, 64], f32)
            uqk32 = wrk.tile([64, 2], f32)
            id8 = wrk.tile([8, 8], f32)
            nc.scalar.copy(pq32[:], pq16[:])
            nc.scalar.copy(pk32[:], pk16[:])
            nc.scalar.copy(uqk32[:], uqk16[:])
            nc.scalar.copy(id8[:], id128[0:8, 0:8])
            onesr = wrk.tile([1, 128], f32)
            onesc = wrk.tile([64, 1], f32)
            nc.vector.memset(onesr[:], 1.0)
            nc.vector.memset(onesc[:], 1.0)

            def eslice(t, g):
                return es[:, (t * 8 + g) * 64:(t * 8 + g + 1) * 64]

            # ---------- Phase B: local Grams -> AllReduce ----------
            G = wrk.tile([64, 1280], f32)
            G16 = wrk.tile([64, 1280], f16)
            esuT = embp.tile([64, NT * 4 * 128], f16)
            with (
                tc.tile_pool(name="gps", bufs=1, space="PSUM") as gps,
                tc.tile_pool(name="tps", bufs=2, space="PSUM") as tps,
            ):
                g_ps = gps.tile([64, 1280], f32)
                for b in range(4):
                    for bu in range(4):
                        col = (b * 4 + bu) * 64
                        for t in range(NT):
                            nc.tensor.matmul(
                                g_ps[:, col:col + 64], eslice(t, 4 + bu),
                                eslice(t, b), start=(t == 0), stop=(t == 3))
                for b in range(4):
                    col = 1024 + b * 64
                    for t in range(NT):
                        nc.tensor.matmul(
                            g_ps[:, col:col + 64], eslice(t, 4 + b),
                            eslice(t, 4 + b), start=(t == 0), stop=(t == 3))
                gl = wrk.tile([64, 1280], f32, tag="gl")
                nc.scalar.copy(gl[:], g_ps[:])
                ar_in = dram.tile([64, 1280], f32)
                ar_out = dram.tile([64, 1280], f32, addr_space="Shared")
                nc.gpsimd.dma_start(ar_in[:], gl[:])
                nc.gpsimd.collective_compute(
                    "AllReduce", mybir.AluOpType.add, replica_groups=RG,
                    ins=[ar_in.opt()], outs=[ar_out.opt()])
                nc.sync.dma_start(G[:], ar_out[:])
                nc.scalar.copy(G16[:], G[:])

                # transposes of the Eu slices for the output matmuls
                for t in range(NT):
                    for bu in range(4):
                        tp = tps.tile([64, 128], f16, tag="tp")
                        nc.tensor.transpose(tp[:], eslice(t, 4 + bu), id128[:])
                        col = (t * 4 + bu) * 128
                        nc.scalar.copy(esuT[:, col:col + 128], tp[:])

            def Gt32(b, bu):
                col = (b * 4 + bu) * 64
                return G[:, col:col + 64]

            def Gt16(b, bu):
                col = (b * 4 + bu) * 64
                return G16[:, col:col + 64]

            def Guu16(b):
                col = 1024 + b * 64
                return G16[:, col:col + 64]

            # ---------- Phase D: cross-path inorm stats (per b) ----------
            bcv = wrk.tile([128, 8], f32)        # per b: (rstd, -mean*rstd)
            with tc.tile_pool(name="stp", bufs=1, space="PSUM") as stp:
                st_ps = stp.tile([1, 8], f32)
                pairs8 = wrk.tile([1, 8], f32, tag="pairs8")
                for b in range(4):
                    g01 = wrk.tile([64, 64], f32, tag="g01")
                    g23 = wrk.tile([64, 64], f32, tag="g23")
                    gsum = wrk.tile([64, 64], f32, tag="gsum")
                    nc.vector.tensor_add(g01[:], Gt32(b, 0), Gt32(b, 1))
                    nc.vector.tensor_add(g23[:], Gt32(b, 2), Gt32(b, 3))
                    nc.vector.tensor_add(gsum[:], g01[:], g23[:])
                    v1_ps = stp.tile([64, 1], f32, tag="v1")
                    nc.tensor.matmul(v1_ps[:], gsum[:], uqk32[:, 1:2])
                    v1 = wrk.tile([64, 1], f32, tag="v1sb")
                    nc.scalar.copy(v1[:], v1_ps[:])
                    nc.tensor.matmul(st_ps[:, 2 * b:2 * b + 1], v1[:],
                                     uqk32[:, 0:1])

                    z_ps = stp.tile([64, 256], f32, tag="z")
                    for bu in range(4):
                        nc.tensor.matmul(z_ps[:, bu * 64:(bu + 1) * 64],
                                         pk32[:], Gt32(b, bu))
                    z_sb = wrk.tile([64, 256], f32, tag="zsb")
                    nc.scalar.copy(z_sb[:], z_ps[:])
                    y_ps = stp.tile([64, 64], f32, tag="y")
                    for bu in range(4):
                        nc.tensor.matmul(y_ps[:], Gt32(b, bu),
                                         z_sb[:, bu * 64:(bu + 1) * 64],
                                         start=(bu == 0), stop=(bu == 3))
                    mq = wrk.tile([64, 64], f32, tag="mq")
                    nc.vector.tensor_mul(mq[:], pq32[:], y_ps[:])
                    mv = wrk.tile([64, 1], f32, tag="mv")
                    nc.vector.reduce_sum(mv[:], mq[:],
                                         axis=mybir.AxisListType.X)
                    nc.tensor.matmul(st_ps[:, 2 * b + 1:2 * b + 2], mv[:],
                                     onesc[:])

                for b in range(4):
                    mean = wrk.tile([1, 1], f32, tag="c0")
                    ex2 = wrk.tile([1, 1], f32, tag="c1")
                    m2 = wrk.tile([1, 1], f32, tag="c2")
                    var = wrk.tile([1, 1], f32, tag="c3")
                    std = wrk.tile([1, 1], f32, tag="c4")
                    rstd = wrk.tile([1, 1], f32, tag="c5")
                    nb = wrk.tile([1, 1], f32, tag="c6")
                    nc.scalar.mul(mean[:], st_ps[:, 2 * b:2 * b + 1],
                                  1.0 / CNT_CROSS)
                    nc.scalar.mul(ex2[:], st_ps[:, 2 * b + 1:2 * b + 2],
                                  1.0 / CNT_CROSS)
                    nc.scalar.square(m2[:], mean[:])
                    nc.vector.tensor_sub(var[:], ex2[:], m2[:])
                    nc.vector.tensor_scalar_add(var[:], var[:], EPS)
                    nc.scalar.activation(std[:], var[:], AF.Sqrt)
                    nc.vector.reciprocal(rstd[:], std[:])
                    nc.vector.tensor_mul(nb[:], mean[:], rstd[:])
                    nc.scalar.copy(pairs8[:, 2 * b:2 * b + 1], rstd[:])
                    nc.scalar.mul(pairs8[:, 2 * b + 1:2 * b + 2], nb[:], -1.0)
                bc_ps = stp.tile([128, 8], f32, tag="bc")
                nc.tensor.matmul(bc_ps[:], onesr[:], pairs8[:])
                nc.scalar.copy(bcv[:], bc_ps[:])

            # ---------- Phase E: self path (per b) ----------
            Weff = wrk.tile([64, 256], f16)
            for b in range(4):
                with tc.tile_pool(name=f"sf{b}", bufs=1, space="PSUM") as sfp:
                    t1_ps = sfp.tile([64, 512], f32, tag="t1")
                    nc.tensor.matmul(t1_ps[:], Guu16(b), wku[:])
                    t1 = wrk.tile([64, 512], f16, tag="t1sb")
                    nc.scalar.copy(t1[:], t1_ps[:])
                    sc_ps = sfp.tile([64, 512], f32, tag="sc")
                    for h in range(H):
                        hb = slice(h * 64, (h + 1) * 64)
                        nc.tensor.matmul(sc_ps[:, hb], wqu[:, hb], t1[:, hb])
                    ss = wrk.tile([64, 16], f32, tag="ss")
                    dump = wrk.tile([64, 64], f32, tag="dump")
                    for h in range(H):
                        hb = slice(h * 64, (h + 1) * 64)
                        nc.scalar.activation(dump[:], sc_ps[:, hb], AF.Copy,
                                             accum_out=ss[:, h:h + 1])
                        nc.scalar.activation(dump[:], sc_ps[:, hb], AF.Square,
                                             accum_out=ss[:, 8 + h:9 + h])
                    tot_ps = sfp.tile([8, 2], f32, tag="tot")
                    nc.tensor.matmul(tot_ps[:, 0:1], ss[:, 0:8], onesc[:])
                    nc.tensor.matmul(tot_ps[:, 1:2], ss[:, 8:16], onesc[:])
                    mean = wrk.tile([8, 1], f32, tag="s0")
                    ex2 = wrk.tile([8, 1], f32, tag="s1")
                    m2 = wrk.tile([8, 1], f32, tag="s2")
                    var = wrk.tile([8, 1], f32, tag="s3")
                    std = wrk.tile([8, 1], f32, tag="s4")
                    pairs = wrk.tile([8, 2], f32, tag="s5")
                    rstd = wrk.tile([8, 1], f32, tag="s6")
                    nb = wrk.tile([8, 1], f32, tag="s7")
                    nc.scalar.mul(mean[:], tot_ps[:, 0:1], 1.0 / CNT_SELF)
                    nc.scalar.mul(ex2[:], tot_ps[:, 1:2], 1.0 / CNT_SELF)
                    nc.scalar.square(m2[:], mean[:])
                    nc.vector.tensor_sub(var[:], ex2[:], m2[:])
                    nc.vector.tensor_scalar_add(var[:], var[:], EPS)
                    nc.scalar.activation(std[:], var[:], AF.Sqrt)
                    nc.vector.reciprocal(rstd[:], std[:])
                    nc.vector.tensor_mul(nb[:], mean[:], rstd[:])
                    nc.scalar.copy(pairs[:, 0:1], rstd[:])
                    nc.scalar.mul(pairs[:, 1:2], nb[:], -1.0)
                    rT_ps = sfp.tile([1, 8], f32, tag="rT")
                    nT_ps = sfp.tile([1, 8], f32, tag="nT")
                    nc.tensor.transpose(rT_ps[:], pairs[:, 0:1], id8[:])
                    nc.tensor.transpose(nT_ps[:], pairs[:, 1:2], id8[:])
                    rn = wrk.tile([1, 16], f32, tag="rn")
                    nc.scalar.copy(rn[:, 0:8], rT_ps[:])
                    nc.scalar.copy(rn[:, 8:16], nT_ps[:])
                    sb_ps = sfp.tile([64, 16], f32, tag="sb")
                    nc.tensor.matmul(sb_ps[:], onesr[0:1, 0:64], rn[:])
                    sbm = wrk.tile([64, 16], f32, tag="sbm")
                    nc.scalar.copy(sbm[:], sb_ps[:])
                    Es = wrk.tile([64, 512], f16, tag="es16")
                    er = wrk.tile([64, 8], f32, tag="er")
                    for h in range(H):
                        hb = slice(h * 64, (h + 1) * 64)
                        nc.scalar.activation(Es[:, hb], sc_ps[:, hb], AF.Exp,
                                             scale=sbm[:, h:h + 1],
                                             bias=sbm[:, 8 + h:9 + h],
                                             accum_out=er[:, h:h + 1])
                    rec = wrk.tile([64, 8], f32, tag="rec")
                    nc.vector.reciprocal(rec[:], er[:])
                    wosc = wrk.tile([64, 512], f16, tag="wosc")
                    for h in range(H):
                        hb = slice(h * 64, (h + 1) * 64)
                        nc.vector.tensor_scalar_mul(wosc[:, hb], woup[:, hb],
                                                    rec[:, h:h + 1])
                    ys_ps = sfp.tile([64, 512], f32, tag="ys")
                    for h in range(H):
                        hb = slice(h * 64, (h + 1) * 64)
                        nc.tensor.matmul(ys_ps[:, hb], Es[:, hb], wosc[:, hb])
                    ys = wrk.tile([64, 512], f16, tag="yssb")
                    nc.scalar.copy(ys[:], ys_ps[:])
                    we_ps = sfp.tile([64, 64], f32, tag="we")
                    for h in range(H):
                        hb = slice(h * 64, (h + 1) * 64)
                        nc.tensor.matmul(we_ps[:], wvut[:, hb], ys[:, hb],
                                         start=(h == 0), stop=(h == 7))
                    nc.scalar.copy(Weff[:, b * 64:(b + 1) * 64], we_ps[:])

            # ---------- Phase F/G: cross path (per b) ----------
            M = wrk.tile([64, 1024], f16)
            for b in range(4):
                with (
                    tc.tile_pool(name=f"cu{b}", bufs=1, space="PSUM") as cup,
                    tc.tile_pool(name=f"cs{b}", bufs=1, space="PSUM") as csp,
                    tc.tile_pool(name=f"cp{b}", bufs=2, space="PSUM") as cpp,
                    tc.tile_pool(name=f"ce{b}", bufs=1) as cep,
                ):
                    U16 = cep.tile([64, 2048], f16, tag="u16")
                    for bu in range(4):
                        u_ps = cup.tile([64, 512], f32, tag="u")
                        nc.tensor.matmul(u_ps[:], Gt16(b, bu), wk[:])
                        nc.scalar.copy(U16[:, bu * 512:(bu + 1) * 512],
                                       u_ps[:])
                    E = cep.tile([128, 4 * 2048], f16, tag="E")
                    r = wrk.tile([128, 4], f32, tag="r")
                    for dc in range(4):
                        s_ps = csp.tile([128, 2048], f32, tag="s")
                        for bu in range(4):
                            nc.tensor.matmul(
                                s_ps[:, bu * 512:(bu + 1) * 512],
                                wq[:, dc * 128:(dc + 1) * 128],
                                U16[:, bu * 512:(bu + 1) * 512])
                        nc.scalar.activation(
                            E[:, dc * 2048:(dc + 1) * 2048], s_ps[:], AF.Exp,
                            scale=bcv[:, 2 * b:2 * b + 1],
                            bias=bcv[:, 2 * b + 1:2 * b + 2],
                            accum_out=r[:, dc:dc + 1])
                    rec = wrk.tile([128, 4], f32, tag="rr")
                    nc.vector.reciprocal(rec[:], r[:])
                    wosc2 = wrk.tile([128, 256], f16, tag="wosc2")
                    for dc in range(4):
                        nc.vector.tensor_scalar_mul(
                            wosc2[:, dc * 64:(dc + 1) * 64],
                            wo[:, dc * 64:(dc + 1) * 64], rec[:, dc:dc + 1])
                    P1 = cep.tile([128, 1024], f16, tag="p1")
                    for bu in range(4):
                        for kvc in range(4):
                            p_ps = cpp.tile([128, 64], f32, tag="p")
                            for dc in range(4):
                                off = dc * 2048 + bu * 512 + kvc * 128
                                nc.tensor.matmul(
                                    p_ps[:], E[:, off:off + 128],
                                    wosc2[:, dc * 64:(dc + 1) * 64],
                                    start=(dc == 0), stop=(dc == 3))
                            col = (bu * 4 + kvc) * 64
                            nc.scalar.copy(P1[:, col:col + 64], p_ps[:])
                    for bu in range(4):
                        m_ps = cup.tile([64, 64], f32, tag="m")
                        for kvc in range(4):
                            col = (bu * 4 + kvc) * 64
                            nc.tensor.matmul(m_ps[:],
                                             wvt[:, kvc * 64:(kvc + 1) * 64],
                                             P1[:, col:col + 64],
                                             start=(kvc == 0), stop=(kvc == 3))
                        nc.scalar.copy(M[:, (b * 4 + bu) * 64:
                                          (b * 4 + bu + 1) * 64], m_ps[:])

            # ---------- Phase H: outputs ----------
            with (
                tc.tile_pool(name="op", bufs=2, space="PSUM") as op,
                tc.tile_pool(name="ob", bufs=4) as ob,
            ):
                for t in range(NT):
                    for b in range(4):
                        o_ps = op.tile([128, 64], f32, tag="o")
                        for bu in range(4):
                            ecol = (t * 4 + bu) * 128
                            nc.tensor.matmul(
                                o_ps[:], esuT[:, ecol:ecol + 128],
                                M[:, (b * 4 + bu) * 64:(b * 4 + bu + 1) * 64],
                                start=(bu == 0), stop=(bu == 3))
                        o16 = ob.tile([128, 64], f16, tag="o16")
                        nc.scalar.copy(o16[:], o_ps[:])
                        nc.sync.dma_start(out_d[t, b], o16[:])
                        ou_ps = op.tile([128, 64], f32, tag="ou")
                        ecol = (t * 4 + b) * 128
                        nc.tensor.matmul(ou_ps[:], esuT[:, ecol:ecol + 128],
                                         Weff[:, b * 64:(b + 1) * 64])
                        ou16 = ob.tile([128, 64], f16, tag="ou16")
                        nc.vector.tensor_copy(ou16[:], ou_ps[:])
                        nc.sync.dma_start(out_d[t, 4 + b], ou16[:])
    nc.compile()
    return nc


# --------------------------------------------------------------------------
# Cached PJRT runner (replaces run_bass_kernel_spmd's per-call jit re-trace)
# --------------------------------------------------------------------------
class _Results:
    def __init__(self, results):
        self.results = results
        self.exec_time_ns = None
        self.mean_exec_time_ns = None
        self.max_exec_time_core_id = None


def _make_runner(nc):
    import jax
    import jax.numpy as jnp
    from jax.sharding import Mesh, NamedSharding, PartitionSpec
    try:
        from jax.sharding import shard_map
    except ImportError:
        from jax.experimental.shard_map import shard_map
    import concourse.mybir as mybir
    from concourse.bass2jax import (_bass_exec_p, fast_dispatch_compile,
                                    install_neuronx_cc_hook,
                                    partition_id_tensor)

    install_neuronx_cc_hook()

    pname = nc.partition_id_tensor.name if nc.partition_id_tensor else None
    in_names, out_names, out_avals = [], [], []
    in_shapes = []
    for alloc in nc.m.functions[0].allocations:
        if not isinstance(alloc, mybir.MemoryLocationSet):
            continue
        name = alloc.memorylocations[0].name
        shape = tuple(alloc.tensor_shape) if alloc.tensor_shape else None
        dtype = mybir.dt.np(alloc.dtype) if alloc.dtype is not None else None
        if alloc.kind == "ExternalInput" and name != pname:
            in_names.append(name)
            in_shapes.append((shape, dtype))
        elif alloc.kind == "ExternalOutput":
            out_names.append(name)
            out_avals.append(jax.core.ShapedArray(shape, dtype))
    n_params = len(in_names)
    n_outs = len(out_avals)
    all_in_names = in_names + out_names + ([pname] if pname else [])

    def _body(*args):
        operands = list(args)
        if pname:
            operands.append(partition_id_tensor())
        outs = _bass_exec_p.bind(
            *operands, out_avals=tuple(out_avals),
            in_names=tuple(all_in_names), out_names=tuple(out_names),
            lowering_input_output_aliases=(), sim_require_finite=True,
            sim_require_nnan=True, nc=nc)
        return tuple(outs)

    devices = jax.devices()[:NC]
    mesh = Mesh(np.asarray(devices), ("core",))
    in_specs = (PartitionSpec("core"),) * (n_params + n_outs)
    out_specs = (PartitionSpec("core"),) * n_outs
    donate = tuple(range(n_params, n_params + n_outs))
    fn = shard_map(_body, mesh=mesh, in_specs=in_specs, out_specs=out_specs,
                   check_rep=False)

    structs = [jax.ShapeDtypeStruct((NC * s[0], *s[1:]), d)
               for (s, d) in in_shapes]
    structs += [jax.ShapeDtypeStruct((NC * a.shape[0], *a.shape[1:]), a.dtype)
                for a in out_avals]
    compiled = fast_dispatch_compile(
        lambda: jax.jit(fn, donate_argnums=donate, keep_unused=True)
        .lower(*structs).compile())

    zshard = tuple([NamedSharding(mesh, PartitionSpec("core"))] * n_outs)
    zfn = jax.jit(
        lambda: tuple(jnp.zeros((NC * a.shape[0], *a.shape[1:]), a.dtype)
                      for a in out_avals),
        out_shardings=zshard)
    state = {"donate": None}

    def run(in_maps):
        concat_in = [np.concatenate([m[name] for m in in_maps], axis=0)
                     for name in in_names]
        # The kernel writes every output element, so the donated output
        # buffers' contents are irrelevant: recycle the previous call's
        # (already fetched) outputs instead of shipping/creating zeros.
        donate_bufs = state["donate"] if state["donate"] is not None else zfn()
        out_arrs = compiled(*concat_in, *donate_bufs)
        fetched = [np.asarray(a) for a in out_arrs]
        state["donate"] = out_arrs
        results = []
        for c in range(NC):
            results.append(
                {name: fetched[i].reshape(NC, *out_avals[i].shape)[c]
                 for i, name in enumerate(out_names)})
        return _Results(results)

    return run


def run_on_device(in_maps, **kwargs):
    kwargs.pop("trace", None)
    if "run" not in _CACHE:
        _CACHE["run"] = _make_runner(_build())
    return _CACHE["run"](in_maps)


# --------------------------------------------------------------------------
# Host-side prep / gather
# --------------------------------------------------------------------------
def _prep_inputs(emb, W_qu, W_ku, W_vu, W_ql2u, W_kl2u, W_vl2u, W_out_u,
                 W_out_l2u):
    emb = np.asarray(emb, np.float32)
    # es[c]: [128, NT*8*64] fp16, block (t, g) at cols (t*8+g)*64
    es_all = (emb.reshape(8, NC, NT, 128, C).transpose(1, 3, 2, 0, 4)
              .reshape(NC, 128, NT * 8 * C)).astype(np.float16)

    w_ou = W_out_u.reshape(C, H, C)
    wvut = np.concatenate([W_vu[:, h * 64:(h + 1) * 64].T for h in range(H)],
                          axis=1)
    woup = np.concatenate([w_ou[:, h, :] for h in range(H)], axis=1)
    wvt_sb = (W_vl2u.T.reshape(4, 128, 64).transpose(1, 0, 2)
              .reshape(128, 256))
    wo_sb = (W_out_l2u.reshape(4, 128, 64).transpose(1, 0, 2)
             .reshape(128, 256))
    pq = W_ql2u @ W_ql2u.T
    pk = W_kl2u @ W_kl2u.T
    id128 = np.eye(128, dtype=np.float32)

    slabs = np.zeros((NC, 64, 1024), np.float32)
    slabs[0, :, 0:512] = W_qu
    slabs[0, :, 512:576] = pq
    slabs[0, :, 576:640] = pk
    slabs[0, :, 640] = W_ql2u.sum(axis=1)
    slabs[0, :, 641] = W_kl2u.sum(axis=1)
    slabs[0, :, 704:832] = id128[0:64]
    slabs[0, :, 832:960] = id128[64:128]
    slabs[1, :, 0:512] = W_ku
    slabs[2, :, 0:512] = wvut
    slabs[3, :, 0:512] = woup
    slabs[4, :, 0:512] = W_ql2u
    slabs[5, :, 0:512] = W_kl2u
    slabs[6, :, 0:256] = wvt_sb[0:64]
    slabs[6, :, 256:512] = wvt_sb[64:128]
    slabs[7, :, 0:256] = wo_sb[0:64]
    slabs[7, :, 256:512] = wo_sb[64:128]
    slabs16 = slabs.astype(np.float16)

    packed = np.empty((NC, 128, NT * 8 * C + 512), np.float16)
    packed[:, :, 0:2048] = es_all
    packed[:, :, 2048:2560] = slabs16.reshape(NC, 128, 512)
    return [{"xin": np.ascontiguousarray(packed[c])} for c in range(NC)]


def kernel(emb, pseudo_label, pseudo_prob_map, W_qu, W_ku, W_vu, W_ql2u,
           W_kl2u, W_vl2u, W_out_u, W_out_l2u, using_SMem, _bass_results=None,
           **_unused):
    del pseudo_label, pseudo_prob_map, using_SMem
    to32 = lambda x: np.asarray(x, np.float32)
    in_maps = _prep_inputs(to32(emb), to32(W_qu), to32(W_ku), to32(W_vu),
                           to32(W_ql2u), to32(W_kl2u), to32(W_vl2u),
                           to32(W_out_u), to32(W_out_l2u))
    if _bass_results is None:
        _bass_results = run_on_device(in_maps).results
    # out[c]: [NT, 8, 128, 64] fp16 -> full[g, c*512 + t*128 + p, k]
    stacked = np.stack([_bass_results[c]["out"] for c in range(NC)])
    out = (stacked.astype(np.float32).transpose(2, 0, 1, 3, 4)
           .reshape(8, N, C))
    return np.ascontiguousarray(out)
